# revision 1
# baseline (speedup 1.0000x reference)
"""RNN-T JointNetwork kernel for 8 Trainium2 NeuronCores.

Math: out[b,t,u,:] = tanh(concat(fe[b,t], gd[b,u])) @ Wj + bj
with fe = f@We+be, gd = g@Wd+bd.

Since tanh acts elementwise and the concat feeds a single GEMM, the joint
GEMM factorizes exactly:
    out[b,t,u,:] = A[b,t,:] + C[b,u,:]
    A = tanh(f@We+be) @ Wj[:Dm]          (per-(b,t) row)
    C = tanh(g@Wd+bd) @ Wj[Dm:] + bj     (per-(b,u) row)
This collapses the 137-GFLOP joint GEMM into two tiny GEMMs plus a
broadcast-add, leaving the kernel bound by the 268 MB output write.

Sharding: 8 cores, core c owns (b = c//2, t-half = c%2) -> a [128,64,V]
output chunk per core (contiguous 33.5 MB).

On-core plan (all fp32):
  - fe^T[m,t] = (We.T @ f^T) via PE (f^T from PE transpose), tanh+bias on ACT
  - gd^T[m,u] likewise
  - A[t,v] (psum = tfT.T @ Wj_top), Cp[u,v] (tgT.T @ Wj_bot + 1x bj)
  - Crep[0:128,v] = Cp stacked twice (selector matmul)
  - per 128-row output tile k (= t-pair 2k,2k+1): broadcast A rows with a
    constant 32-row selector bank (32-aligned slices of A as matmul rhs),
    add Crep on DVE (half 0) / replicate Cp on PE + copy on ACT (half 1),
    DMA 512 KB contiguous per tile.
"""

import sys

sys.path.insert(0, "/opt/trn_rl_repo")

import numpy as np

import concourse.bacc as bacc
import concourse.mybir as mybir
import concourse.tile as tile
from concourse.bass_utils import run_bass_kernel_spmd
from concourse.masks import make_identity

B, T, U = 4, 256, 64
D = 512  # DE = DD = DM
V = 1024
TC = 128  # t rows per core
NCORES = 8
FP32 = mybir.dt.float32
BF16 = mybir.dt.bfloat16
TANH = mybir.ActivationFunctionType.Tanh

_cache = {}


def _build_nc():
    nc = bacc.Bacc("TRN2", target_bir_lowering=False)

    f_d = nc.dram_tensor("f_c", [TC, D], FP32, kind="ExternalInput")
    g_d = nc.dram_tensor("g_c", [U, D], FP32, kind="ExternalInput")
    We_d = nc.dram_tensor("We", [D, D], FP32, kind="ExternalInput")
    be_d = nc.dram_tensor("be", [D], FP32, kind="ExternalInput")
    Wd_d = nc.dram_tensor("Wd", [D, D], FP32, kind="ExternalInput")
    bd_d = nc.dram_tensor("bd", [D], FP32, kind="ExternalInput")
    Wj_d = nc.dram_tensor("Wj", [2 * D, V], FP32, kind="ExternalInput")
    bj_d = nc.dram_tensor("bj", [V], FP32, kind="ExternalInput")
    out_d = nc.dram_tensor("out", [TC * U, V], FP32, kind="ExternalOutput")

    with tile.TileContext(nc) as tc:
        with (
            tc.tile_pool(name="const", bufs=1) as cp,
            tc.tile_pool(name="wts", bufs=1) as wp,
        ):
            # ---- constants ----
            ident = cp.tile([128, 128], FP32, tag="ident")
            make_identity(nc, ident[:])

            # selrep[u, j] = 1 iff j%64 == u  ([I64 | I64])
            selrep = cp.tile([64, 128], FP32, tag="selrep")
            nc.gpsimd.memset(selrep[:], 0.0)
            nc.gpsimd.affine_select(
                out=selrep[:].rearrange("p (a b) -> p a b", a=2),
                in_=selrep[:].rearrange("p (a b) -> p a b", a=2),
                compare_op=mybir.AluOpType.not_equal,
                fill=1.0,
                base=0,
                pattern=[[0, 2], [-1, 64]],
                channel_multiplier=1,
            )

            # sel32[32q + t', 128i + 64jh + jl] = 1 iff t' == 2i + jh
            # (identical pattern in each 32-partition strip q). bf16: the
            # selector is 0/1 so bf16 matmuls against bf16 hi/lo terms of A
            # select exactly, at 1 col/cycle instead of fp32's multi-pass.
            sel32 = cp.tile([128, 16 * 128], BF16, tag="sel32")
            nc.gpsimd.memset(sel32[:], 0.0)
            for q in range(4):
                sl = sel32[32 * q : 32 * q + 32, :]
                nc.gpsimd.affine_select(
                    out=sl.rearrange("p (i a b) -> p i a b", i=16, a=2),
                    in_=sl.rearrange("p (i a b) -> p i a b", i=16, a=2),
                    compare_op=mybir.AluOpType.not_equal,
                    fill=1.0,
                    base=0,
                    pattern=[[-2, 16], [-1, 2], [0, 64]],
                    channel_multiplier=1,
                )

            ones1 = cp.tile([1, 64], FP32, tag="ones1")
            nc.gpsimd.memset(ones1[:], 1.0)

            # dup_hi/dup_lo: build AHL = [Ahi(0:32); Alo(0:32); Ahi(32:64);
            # Alo(32:64)] per 64-row half. j = 64*jh2 + 32*jm + jl.
            # dup_hi[t', j] = 1 iff jm==0 and t' == 32*jh2 + jl
            # dup_lo[t', j] = 1 iff jm==1 and t' == 32*jh2 + jl
            dup_hi = cp.tile([128, 128], BF16, tag="dup_hi")
            dup_lo = cp.tile([128, 128], BF16, tag="dup_lo")
            for tile_, base in ((dup_hi, 0), (dup_lo, 64)):
                nc.gpsimd.memset(tile_[:], 0.0)
                for s in range(2):
                    sl = tile_[64 * s : 64 * s + 64, :]
                    nc.gpsimd.affine_select(
                        out=sl.rearrange("p (a b c) -> p a b c", a=2, b=2),
                        in_=sl.rearrange("p (a b c) -> p a b c", a=2, b=2),
                        compare_op=mybir.AluOpType.not_equal,
                        fill=1.0,
                        base=base,
                        pattern=[[-32, 2], [-64 if base else 64, 2], [-1, 32]],
                        channel_multiplier=1,
                    )

            # ---- persistent operands ----
            f_sb = wp.tile([TC, D], FP32, tag="f")
            g_sb = wp.tile([U, D], FP32, tag="g")
            We_sb = [wp.tile([128, D], FP32, tag=f"We{c}", name=f"We{c}") for c in range(4)]
            Wd_sb = [wp.tile([128, D], FP32, tag=f"Wd{c}", name=f"Wd{c}") for c in range(4)]
            Wj_sb = [wp.tile([128, V], FP32, tag=f"Wj{c}", name=f"Wj{c}") for c in range(8)]
            be_sb = [wp.tile([128, 1], FP32, tag=f"be{c}", name=f"be{c}") for c in range(4)]
            bd_sb = [wp.tile([128, 1], FP32, tag=f"bd{c}", name=f"bd{c}") for c in range(4)]
            bj_sb = wp.tile([1, V], FP32, tag="bj")
            fT = [wp.tile([128, TC], FP32, tag=f"fT{c}", name=f"fT{c}") for c in range(4)]
            gT = [wp.tile([128, U], FP32, tag=f"gT{c}", name=f"gT{c}") for c in range(4)]
            tfT = [wp.tile([128, TC], FP32, tag=f"tfT{c}", name=f"tfT{c}") for c in range(4)]
            tgT = [wp.tile([128, U], FP32, tag=f"tgT{c}", name=f"tgT{c}") for c in range(4)]
            A_sb = wp.tile([TC, V], FP32, tag="A")
            A_hi = wp.tile([TC, V], BF16, tag="A_hi")
            A_lo = wp.tile([TC, V], BF16, tag="A_lo")
            A_tmp = wp.tile([TC, V], FP32, tag="A_tmp")
            AHL = [wp.tile([128, V], BF16, tag=f"AHL{h}", name=f"AHL{h}") for h in range(2)]
            Cp = wp.tile([U, V], FP32, tag="Cp")
            Crep = wp.tile([128, V], FP32, tag="Crep")

            nc.sync.dma_start(f_sb[:], f_d[:])
            nc.sync.dma_start(g_sb[:], g_d[:])
            for c in range(4):
                nc.sync.dma_start(We_sb[c][:], We_d[c * 128 : (c + 1) * 128, :])
                nc.sync.dma_start(Wd_sb[c][:], Wd_d[c * 128 : (c + 1) * 128, :])
                nc.sync.dma_start(
                    be_sb[c][:],
                    be_d[c * 128 : (c + 1) * 128].rearrange("(p o) -> p o", o=1),
                )
                nc.sync.dma_start(
                    bd_sb[c][:],
                    bd_d[c * 128 : (c + 1) * 128].rearrange("(p o) -> p o", o=1),
                )
            for c in range(8):
                nc.sync.dma_start(Wj_sb[c][:], Wj_d[c * 128 : (c + 1) * 128, :])
            nc.sync.dma_start(bj_sb[:], bj_d.rearrange("(p v) -> p v", p=1))

            # ---- prologue: A, Cp, Crep ----
            with tc.tile_pool(name="pp", bufs=4, space="PSUM") as pp:
                for c in range(4):
                    pt = pp.tile([128, 128], FP32, tag="pps")
                    nc.tensor.transpose(
                        pt[:], f_sb[:, c * 128 : (c + 1) * 128], ident[:]
                    )
                    nc.vector.tensor_copy(fT[c][:], pt[:])
                for c in range(4):
                    pt = pp.tile([128, U], FP32, tag="pps")
                    nc.tensor.transpose(
                        pt[:], g_sb[:, c * 128 : (c + 1) * 128], ident[0:64, 0:64]
                    )
                    nc.vector.tensor_copy(gT[c][:], pt[:])

                for mc in range(4):
                    ms = slice(mc * 128, (mc + 1) * 128)
                    ps = pp.tile([128, TC], FP32, tag="pps")
                    for dc in range(4):
                        nc.tensor.matmul(
                            ps[:],
                            We_sb[dc][:, ms],
                            fT[dc][:],
                            start=(dc == 0),
                            stop=(dc == 3),
                        )
                    nc.scalar.activation(
                        tfT[mc][:], ps[:], TANH, bias=be_sb[mc][:, 0:1]
                    )
                for mc in range(4):
                    ms = slice(mc * 128, (mc + 1) * 128)
                    ps = pp.tile([128, U], FP32, tag="pps")
                    for dc in range(4):
                        nc.tensor.matmul(
                            ps[:],
                            Wd_sb[dc][:, ms],
                            gT[dc][:],
                            start=(dc == 0),
                            stop=(dc == 3),
                        )
                    nc.scalar.activation(
                        tgT[mc][:], ps[:], TANH, bias=bd_sb[mc][:, 0:1]
                    )

                for vh in range(2):
                    vs = slice(vh * 512, (vh + 1) * 512)
                    ps = pp.tile([128, 512], FP32, tag="pps")
                    for mc in range(4):
                        nc.tensor.matmul(
                            ps[:],
                            tfT[mc][:],
                            Wj_sb[mc][:, vs],
                            start=(mc == 0),
                            stop=(mc == 3),
                        )
                    nc.vector.tensor_copy(A_sb[:, vs], ps[:])
                for vh in range(2):
                    vs = slice(vh * 512, (vh + 1) * 512)
                    ps = pp.tile([64, 512], FP32, tag="pps")
                    for mc in range(4):
                        nc.tensor.matmul(
                            ps[:],
                            tgT[mc][:],
                            Wj_sb[4 + mc][:, vs],
                            start=(mc == 0),
                            stop=False,
                        )
                    nc.tensor.matmul(
                        ps[:], ones1[:], bj_sb[:, vs], start=False, stop=True
                    )
                    nc.scalar.copy(Cp[:, vs], ps[:])
                for vh in range(2):
                    vs = slice(vh * 512, (vh + 1) * 512)
                    ps = pp.tile([128, 512], FP32, tag="pps")
                    nc.tensor.matmul(ps[:], selrep[:], Cp[:, vs], start=True, stop=True)
                    nc.vector.tensor_copy(Crep[:, vs], ps[:])

                # exact-ish two-term bf16 split A = A_hi + A_lo + O(2^-17),
                # done per 64-row half so AHL[0] (tiles 0..31) unblocks early;
                # AHL[h] = [Ahi(64h+0:32); Alo(same); Ahi(64h+32:64); Alo(same)]
                # via dup-selector matmuls (bf16 0/1 select, exact)
                for h in range(2):
                    hs = slice(64 * h, 64 * h + 64)
                    nc.vector.tensor_copy(A_hi[hs, :], A_sb[hs, :])
                    nc.vector.tensor_copy(A_tmp[hs, :], A_hi[hs, :])
                    nc.vector.tensor_sub(A_tmp[hs, :], A_sb[hs, :], A_tmp[hs, :])
                    nc.vector.tensor_copy(A_lo[hs, :], A_tmp[hs, :])
                    for vh in range(2):
                        vs = slice(vh * 512, (vh + 1) * 512)
                        ps = pp.tile([128, 512], FP32, tag="pps")
                        nc.tensor.matmul(
                            ps[:], dup_hi[hs, :], A_hi[hs, vs],
                            start=True, stop=False, tile_position=(64 * h, 0),
                        )
                        nc.tensor.matmul(
                            ps[:], dup_lo[hs, :], A_lo[hs, vs],
                            start=False, stop=True, tile_position=(64 * h, 0),
                        )
                        nc.vector.tensor_copy(AHL[h][:, vs], ps[:])

            # ---- main loop: 64 output tiles of [128, 1024] ----
            with (
                tc.tile_pool(name="po", bufs=4, space="PSUM") as po,
                tc.tile_pool(name="ob", bufs=8) as ob,
            ):
                for k in range(64):
                    q, i = k // 16, k % 16
                    h, r = q // 2, q % 2
                    rs = slice(64 * r, 64 * r + 64)
                    lhs_sel = sel32[rs, i * 128 : (i + 1) * 128]
                    psO = po.tile([128, V], FP32, tag="psO")
                    out_sb = ob.tile([128, V], FP32, tag="out")
                    # A broadcast (hi+lo packed, K=64) on PE, one MM per bank
                    for vh in range(2):
                        vs = slice(vh * 512, (vh + 1) * 512)
                        nc.tensor.matmul(
                            psO[:, vs], lhs_sel, AHL[h][rs, vs],
                            start=True, stop=True, tile_position=(64 * r, 0),
                        )
                    # single full-width DVE add does C + the PSUM->SBUF move
                    nc.vector.tensor_add(out_sb[:], psO[:], Crep[:])
                    nc.sync.dma_start(
                        out_d[k * 128 : (k + 1) * 128, :], out_sb[:]
                    )

    nc.compile()
    return nc


def kernel(f, g, We, be, Wd, bd, Wj, bj):
    if "nc" not in _cache:
        _cache["nc"] = _build_nc()
    nc = _cache["nc"]

    cast = lambda x: np.ascontiguousarray(np.asarray(x), dtype=np.float32)
    f, g = cast(f), cast(g)
    shared = {
        "We": cast(We), "be": cast(be), "Wd": cast(Wd), "bd": cast(bd),
        "Wj": cast(Wj), "bj": cast(bj),
    }
    in_maps = []
    for c in range(NCORES):
        b, th = c // 2, c % 2
        in_maps.append(
            {
                "f_c": np.ascontiguousarray(f[b, th * TC : (th + 1) * TC, :]),
                "g_c": np.ascontiguousarray(g[b]),
                **shared,
            }
        )
    res = run_bass_kernel_spmd(nc, in_maps, list(range(NCORES)))
    kernel._last_results = res

    out = np.empty((B, T, U, V), np.float32)
    for c in range(NCORES):
        b, th = c // 2, c % 2
        out[b, th * TC : (th + 1) * TC] = res.results[c]["out"].reshape(TC, U, V)
    return out



# revision 4
# speedup vs baseline: 1.6484x; 1.6484x over previous
"""RNN-T JointNetwork kernel for 8 Trainium2 NeuronCores.

Math: out[b,t,u,:] = tanh(concat(fe[b,t], gd[b,u])) @ Wj + bj
with fe = f@We+be, gd = g@Wd+bd.

tanh is elementwise and the concat feeds one GEMM, so the joint GEMM
factorizes exactly:
    out[b,t,u,:] = A[b,t,:] + C[b,u,:]
    A = tanh(f@We+be) @ Wj[:Dm]
    C = tanh(g@Wd+bd) @ Wj[Dm:] + bj
leaving the kernel bound by the output write (fp16: 16 MB/core).

Sharding: core c owns (b = c//2, t-half = c%2) -> [128,64,V] output chunk.

On-core plan (fp16 data, fp32 PSUM):
  - fT/gT via DMA-transpose; feT/gdT on PE; tanh+bias on ACT -> tfT/tgT
  - ACP[h] (h = t-half of the core's 128 rows): [128,V] fp16 tile with
    partitions 0:64 = C rows (all u) and 64:128 = A rows (t in half h),
    both computed directly into one PSUM tile via column-tiled matmuls.
  - main loop, per output tile k (= t-pair 2k,2k+1; 128 DRAM rows):
    psO[:,vh] = L_k.T @ ACP[h][:,vh] for the two 512-col banks, where
    L_k is a host-built 0/1 stationary selecting (A row, C row) per
    output row -> A-broadcast + C-add in a single PE pass.
    Movers alternate per tile: DVE copy / ACT copy PSUM->SBUF fp16.
    Output DMA: 1 MB per 4 tiles.
Host: casts inputs to fp16, builds L, upcasts output to fp32 on gather.
"""

import sys

sys.path.insert(0, "/opt/trn_rl_repo")

import numpy as np

import concourse.bacc as bacc
import concourse.mybir as mybir
import concourse.tile as tile
from concourse.bass_utils import run_bass_kernel_spmd

B, T, U = 4, 256, 64
D = 512  # DE = DD = DM
V = 1024
TC = 128  # t rows per core
NCORES = 8
FP32 = mybir.dt.float32
FP16 = mybir.dt.float16
TANH = mybir.ActivationFunctionType.Tanh

_cache = {}


def _build_nc():
    nc = bacc.Bacc("TRN2", target_bir_lowering=False)

    f_d = nc.dram_tensor("f_c", [TC, D], FP16, kind="ExternalInput")
    g_d = nc.dram_tensor("g_c", [U, D], FP16, kind="ExternalInput")
    We_d = nc.dram_tensor("We", [D, D], FP16, kind="ExternalInput")
    Wd_d = nc.dram_tensor("Wd", [D, D], FP16, kind="ExternalInput")
    Wj_d = nc.dram_tensor("Wj", [2 * D, V], FP16, kind="ExternalInput")
    L_d = nc.dram_tensor("Lsel", [128, 32 * 128], FP16, kind="ExternalInput")
    bias_d = nc.dram_tensor("bias8", [128, 8], FP32, kind="ExternalInput")
    bj_d = nc.dram_tensor("bj", [1, V], FP16, kind="ExternalInput")
    out_d = nc.dram_tensor("out", [TC * U, V], FP16, kind="ExternalOutput")

    with tile.TileContext(nc) as tc:
        with tc.tile_pool(name="wts", bufs=1) as wp:
            # ---- persistent SBUF ----
            fT = [wp.tile([128, TC], FP16, tag=f"fT{c}", name=f"fT{c}") for c in range(4)]
            gT = [wp.tile([128, U], FP16, tag=f"gT{c}", name=f"gT{c}") for c in range(4)]
            We_sb = [wp.tile([128, D], FP16, tag=f"We{c}", name=f"We{c}") for c in range(4)]
            Wd_sb = [wp.tile([128, D], FP16, tag=f"Wd{c}", name=f"Wd{c}") for c in range(4)]
            Wj_sb = [wp.tile([128, V], FP16, tag=f"Wj{c}", name=f"Wj{c}") for c in range(8)]
            L_sb = wp.tile([128, 32 * 128], FP16, tag="Lsel")
            bias_sb = wp.tile([128, 8], FP32, tag="bias8")
            bj_sb = wp.tile([1, V], FP16, tag="bj")
            ones1 = wp.tile([1, U], FP16, tag="ones1")
            tfT = [wp.tile([128, TC], FP16, tag=f"tfT{c}", name=f"tfT{c}") for c in range(4)]
            tgT = [wp.tile([128, U], FP16, tag=f"tgT{c}", name=f"tgT{c}") for c in range(4)]
            ACP = [wp.tile([128, V], FP16, tag=f"ACP{h}", name=f"ACP{h}") for h in range(2)]

            # ---- input DMAs (split across the two HWDGE rings) ----
            for c in range(4):
                nc.sync.dma_start(
                    fT[c][:], f_d[:, c * 128 : (c + 1) * 128], transpose=True
                )
            for c in range(4):
                nc.sync.dma_start(We_sb[c][:], We_d[c * 128 : (c + 1) * 128, :])
            for c in range(4):
                nc.scalar.dma_start(
                    gT[c][:], g_d[:, c * 128 : (c + 1) * 128], transpose=True
                )
            for c in range(4):
                nc.scalar.dma_start(Wd_sb[c][:], Wd_d[c * 128 : (c + 1) * 128, :])
            # Wj top half (A) first on sync; bottom (C) on scalar
            for c in range(4):
                nc.sync.dma_start(Wj_sb[c][:], Wj_d[c * 128 : (c + 1) * 128, :])
                nc.scalar.dma_start(
                    Wj_sb[4 + c][:], Wj_d[(4 + c) * 128 : (5 + c) * 128, :]
                )
            nc.scalar.dma_start(bias_sb[:], bias_d[:])
            nc.scalar.dma_start(bj_sb[:], bj_d[:])
            nc.sync.dma_start(L_sb[:], L_d[:])
            nc.gpsimd.memset(ones1[:], 1.0)

            # ---- prologue: tfT, tgT, ACP ----
            with tc.tile_pool(name="pp", bufs=4, space="PSUM") as pp:
                for mc in range(4):
                    ms = slice(mc * 128, (mc + 1) * 128)
                    ps = pp.tile([128, TC], FP32, tag="pps")
                    for dc in range(4):
                        nc.tensor.matmul(
                            ps[:],
                            We_sb[dc][:, ms],
                            fT[dc][:],
                            start=(dc == 0),
                            stop=(dc == 3),
                        )
                    nc.scalar.activation(
                        tfT[mc][:], ps[:], TANH, bias=bias_sb[:, mc : mc + 1]
                    )
                for mc in range(4):
                    ms = slice(mc * 128, (mc + 1) * 128)
                    ps = pp.tile([128, U], FP32, tag="pps")
                    for dc in range(4):
                        nc.tensor.matmul(
                            ps[:],
                            Wd_sb[dc][:, ms],
                            gT[dc][:],
                            start=(dc == 0),
                            stop=(dc == 3),
                        )
                    nc.scalar.activation(
                        tgT[mc][:], ps[:], TANH, bias=bias_sb[:, 4 + mc : 5 + mc]
                    )

                # ACP[h]: partitions 0:64 = C (all u), 64:128 = A rows 64h..64h+63
                nmv = 0
                for h in range(2):
                    hs = slice(64 * h, 64 * h + 64)
                    for vh in range(2):
                        vs = slice(vh * 512, (vh + 1) * 512)
                        ps = pp.tile([128, 512], FP32, tag="pps")
                        for mc in range(4):
                            nc.tensor.matmul(
                                ps[0:64, :],
                                tgT[mc][:],
                                Wj_sb[4 + mc][:, vs],
                                start=(mc == 0),
                                stop=False,
                            )
                        nc.tensor.matmul(
                            ps[0:64, :], ones1[:], bj_sb[:, vs], start=False, stop=True
                        )
                        for mc in range(4):
                            nc.tensor.matmul(
                                ps[64:128, :],
                                tfT[mc][:, hs],
                                Wj_sb[mc][:, vs],
                                start=(mc == 0),
                                stop=(mc == 3),
                                tile_position=(0, 64),
                            )
                        if nmv % 2 == 0:
                            nc.vector.tensor_copy(ACP[h][:, vs], ps[:])
                        else:
                            nc.scalar.copy(ACP[h][:, vs], ps[:])
                        nmv += 1

            # ---- main loop: 64 output tiles of [128, 1024], 4 per DMA ----
            with (
                tc.tile_pool(name="po", bufs=4, space="PSUM") as po,
                tc.tile_pool(name="ob", bufs=3) as ob,
            ):
                grp = None
                for k in range(64):
                    h, kk = k // 32, k % 32
                    if k % 4 == 0:
                        grp = ob.tile([128, 4 * V], FP16, tag="grp")
                    psO = po.tile([128, V], FP32, tag="psO")
                    Lk = L_sb[:, kk * 128 : (kk + 1) * 128]
                    for vh in range(2):
                        vs = slice(vh * 512, (vh + 1) * 512)
                        nc.tensor.matmul(
                            psO[:, vs], Lk, ACP[h][:, vs], start=True, stop=True
                        )
                    dst = grp[:, (k % 4) * V : (k % 4 + 1) * V]
                    if k % 2 == 0:
                        nc.vector.tensor_copy(dst, psO[:])
                    else:
                        nc.scalar.copy(dst, psO[:])
                    if k % 4 == 3:
                        g0 = (k // 4) * 512
                        nc.sync.dma_start(
                            out_d[g0 : g0 + 512, :].rearrange(
                                "(s p) v -> p s v", p=128
                            ),
                            grp[:],
                        )

    nc.compile()
    return nc


def _host_consts():
    """Selector stationaries L_k (k=0..31): L[kap, 128k+j] so that
    (L_k.T @ ACP)[j,:] = C[j%64,:] + A[64h + 2k + j//64, :]."""
    L = np.zeros((128, 32, 128), np.float16)
    j = np.arange(128)
    u = j % 64
    hi = j // 64
    L[u, :, j] = 1.0
    for kk in range(32):
        L[64 + 2 * kk + hi, kk, j] = 1.0
    return np.ascontiguousarray(L.reshape(128, 32 * 128))


def kernel(f, g, We, be, Wd, bd, Wj, bj):
    if "nc" not in _cache:
        _cache["nc"] = _build_nc()
    nc = _cache["nc"]

    c16 = lambda x: np.ascontiguousarray(np.asarray(x), dtype=np.float16)
    f16, g16 = c16(f), c16(g)
    be32 = np.asarray(be, np.float32).reshape(4, 128).T
    bd32 = np.asarray(bd, np.float32).reshape(4, 128).T
    bias8 = np.ascontiguousarray(
        np.concatenate([be32, bd32], axis=1), dtype=np.float32
    )
    shared = {
        "We": c16(We),
        "Wd": c16(Wd),
        "Wj": c16(Wj),
        "bj": c16(bj).reshape(1, V),
        "bias8": bias8,
        "Lsel": _host_consts(),
    }
    in_maps = []
    for c in range(NCORES):
        b, th = c // 2, c % 2
        in_maps.append(
            {
                "f_c": np.ascontiguousarray(f16[b, th * TC : (th + 1) * TC, :]),
                "g_c": np.ascontiguousarray(g16[b]),
                **shared,
            }
        )
    res = run_bass_kernel_spmd(nc, in_maps, list(range(NCORES)))
    kernel._last_results = res

    out = np.empty((B, T, U, V), np.float32)
    for c in range(NCORES):
        b, th = c // 2, c % 2
        out[b, th * TC : (th + 1) * TC] = res.results[c]["out"].reshape(TC, U, V)
    return out


# revision 6
# speedup vs baseline: 1.6803x; 1.0194x over previous
"""RNN-T JointNetwork kernel for 8 Trainium2 NeuronCores.

Math: out[b,t,u,:] = tanh(concat(fe[b,t], gd[b,u])) @ Wj + bj
with fe = f@We+be, gd = g@Wd+bd.

tanh is elementwise and the concat feeds one GEMM, so the joint GEMM
factorizes exactly:
    out[b,t,u,:] = A[b,t,:] + C[b,u,:]
    A = tanh(f@We+be) @ Wj[:Dm]
    C = tanh(g@Wd+bd) @ Wj[Dm:] + bj
leaving the kernel bound by the output write (fp16: 16 MB/core).

Sharding: core c owns (b = c//2, t-half = c%2) -> [128,64,V] output chunk.

On-core plan (fp16 data, fp32 PSUM):
  - f/g straight-DMA'd, transposed on PE (fp16 psum passthrough);
    feT/gdT on PE; tanh+bias on ACT -> tfT/tgT
  - ACP[h] (h = t-half of the core's 128 rows): [128,V] fp16 tile with
    partitions 0:64 = C rows (all u) and 64:128 = A rows (t in half h),
    computed into one PSUM tile with C (cols 0:64) and A (cols 64:128)
    matmuls interleaved so the two PE column-groups run concurrently.
  - main loop, per output tile k (= t-pair 2k,2k+1; 128 DRAM rows):
    psO = L_k.T @ ACP[h], where L_k is a host-built 0/1 stationary
    selecting (A row, C row) per output row -> A-broadcast + C-add in
    a single PE pass.  Movers alternate per tile: DVE / ACT copy
    PSUM->SBUF fp16.  Output DMA: 1 MB per 4 tiles.
Host: casts inputs to fp16, builds L/identity, upcasts output on gather.
"""

import sys

sys.path.insert(0, "/opt/trn_rl_repo")

import numpy as np

import concourse.bacc as bacc
import concourse.mybir as mybir
import concourse.tile as tile
from concourse.bass_utils import run_bass_kernel_spmd

B, T, U = 4, 256, 64
D = 512  # DE = DD = DM
V = 1024
TC = 128  # t rows per core
NCORES = 8
FP32 = mybir.dt.float32
FP16 = mybir.dt.float16
TANH = mybir.ActivationFunctionType.Tanh
WIDE = False  # N=1024 matmul per tile fails the walrus ISA check (1 bank/MM)

_cache = {}


def _build_nc():
    nc = bacc.Bacc("TRN2", target_bir_lowering=False)

    f_d = nc.dram_tensor("f_c", [TC, D], FP16, kind="ExternalInput")
    g_d = nc.dram_tensor("g_c", [U, D], FP16, kind="ExternalInput")
    We_d = nc.dram_tensor("We", [D, D], FP16, kind="ExternalInput")
    Wd_d = nc.dram_tensor("Wd", [D, D], FP16, kind="ExternalInput")
    Wj_d = nc.dram_tensor("Wj", [2 * D, V], FP16, kind="ExternalInput")
    # consts: 32 L selectors + identity(128) + ones row
    L_d = nc.dram_tensor("Lsel", [128, 34 * 128], FP16, kind="ExternalInput")
    bias_d = nc.dram_tensor("bias8", [128, 8], FP32, kind="ExternalInput")
    bj_d = nc.dram_tensor("bj", [1, V], FP16, kind="ExternalInput")
    out_d = nc.dram_tensor("out", [TC * U, V], FP16, kind="ExternalOutput")

    with tile.TileContext(nc) as tc:
        with tc.tile_pool(name="wts", bufs=1) as wp:
            # ---- persistent SBUF ----
            f_sb = wp.tile([TC, D], FP16, tag="f")
            g_sb = wp.tile([U, D], FP16, tag="g")
            We_sb = wp.tile([128, 4 * D], FP16, tag="We")
            Wd_sb = wp.tile([128, 4 * D], FP16, tag="Wd")
            Wj_sb = wp.tile([128, 8 * V], FP16, tag="Wj")
            L_sb = wp.tile([128, 34 * 128], FP16, tag="Lsel")
            bias_sb = wp.tile([128, 8], FP32, tag="bias8")
            bj_sb = wp.tile([1, V], FP16, tag="bj")
            fT = [wp.tile([128, TC], FP16, tag=f"fT{c}", name=f"fT{c}") for c in range(4)]
            gT = [wp.tile([128, U], FP16, tag=f"gT{c}", name=f"gT{c}") for c in range(4)]
            tfT = [wp.tile([128, TC], FP16, tag=f"tfT{c}", name=f"tfT{c}") for c in range(4)]
            tgT = [wp.tile([128, U], FP16, tag=f"tgT{c}", name=f"tgT{c}") for c in range(4)]
            ACP = [wp.tile([128, V], FP16, tag=f"ACP{h}", name=f"ACP{h}") for h in range(2)]

            ident = L_sb[:, 32 * 128 : 33 * 128]
            ones1 = L_sb[0:1, 33 * 128 : 33 * 128 + U]

            # ---- input DMAs: few big transfers, split across HWDGE rings
            # sync ring: f, We, Wj v-half 0, L  (output DMAs follow in FIFO)
            # scalar ring: g, bias, bj, Wd, Wj v-half 1
            nc.sync.dma_start(f_sb[:], f_d[:])
            nc.sync.dma_start(
                We_sb[:], We_d.rearrange("(c p) m -> p c m", p=128)
            )
            nc.scalar.dma_start(g_sb[:], g_d[:])
            nc.scalar.dma_start(bias_sb[:], bias_d[:])
            nc.scalar.dma_start(bj_sb[:], bj_d[:])
            nc.scalar.dma_start(
                Wd_sb[:], Wd_d.rearrange("(c p) m -> p c m", p=128)
            )
            # Wj_sb layout: [128, c*V + v] (chunk-major); DMA by v-half
            for vh in range(2):
                eng = nc.sync if vh == 0 else nc.scalar
                eng.dma_start(
                    Wj_sb[:]
                    .rearrange("p (c v) -> p c v", c=8)[:, :, vh * 512 : (vh + 1) * 512],
                    Wj_d.rearrange("(c p) v -> p c v", p=128)[
                        :, :, vh * 512 : (vh + 1) * 512
                    ],
                )
            nc.sync.dma_start(L_sb[:], L_d[:])

            # ---- prologue ----
            with tc.tile_pool(name="pp", bufs=4, space="PSUM") as pp:
                # transposes on PE (fp16 passthrough), copies split DVE/ACT
                for c in range(4):
                    pt = pp.tile([128, TC], FP16, tag="ppt")
                    nc.tensor.transpose(
                        pt[:], f_sb[:, c * 128 : (c + 1) * 128], ident
                    )
                    if c % 2 == 0:
                        nc.vector.tensor_copy(fT[c][:], pt[:])
                    else:
                        nc.scalar.copy(fT[c][:], pt[:])
                for c in range(4):
                    pt = pp.tile([128, U], FP16, tag="ppt")
                    nc.tensor.transpose(
                        pt[:], g_sb[:, c * 128 : (c + 1) * 128], ident[0:64, 0:64]
                    )
                    if c % 2 == 0:
                        nc.vector.tensor_copy(gT[c][:], pt[:])
                    else:
                        nc.scalar.copy(gT[c][:], pt[:])

                for mc in range(4):
                    ms = slice(mc * 128, (mc + 1) * 128)
                    ps = pp.tile([128, TC], FP32, tag="pps")
                    for dc in range(4):
                        nc.tensor.matmul(
                            ps[:],
                            We_sb[:, dc * D : (dc + 1) * D][:, ms],
                            fT[dc][:],
                            start=(dc == 0),
                            stop=(dc == 3),
                        )
                    nc.scalar.activation(
                        tfT[mc][:], ps[:], TANH, bias=bias_sb[:, mc : mc + 1]
                    )
                for mc in range(4):
                    ms = slice(mc * 128, (mc + 1) * 128)
                    ps = pp.tile([128, U], FP32, tag="pps")
                    for dc in range(4):
                        nc.tensor.matmul(
                            ps[:],
                            Wd_sb[:, dc * D : (dc + 1) * D][:, ms],
                            gT[dc][:],
                            start=(dc == 0),
                            stop=(dc == 3),
                        )
                    nc.scalar.activation(
                        tgT[mc][:], ps[:], TANH, bias=bias_sb[:, 4 + mc : 5 + mc]
                    )

                # ACP[h]: partitions 0:64 = C (all u), 64:128 = A rows of half h.
                # C matmuls hit PE col-group 0:64, A matmuls col-group 64:128 —
                # interleave them so the groups execute concurrently.
                nmv = 0
                for h in range(2):
                    hs = slice(64 * h, 64 * h + 64)
                    for vh in range(2):
                        vs = slice(vh * 512, (vh + 1) * 512)
                        Wjc = lambda c: Wj_sb[:, c * V : (c + 1) * V][:, vs]
                        ps = pp.tile([128, 512], FP32, tag="pps")
                        for mc in range(4):
                            nc.tensor.matmul(
                                ps[0:64, :],
                                tgT[mc][:],
                                Wjc(4 + mc),
                                start=(mc == 0),
                                stop=False,
                            )
                            nc.tensor.matmul(
                                ps[64:128, :],
                                tfT[mc][:, hs],
                                Wjc(mc),
                                start=(mc == 0),
                                stop=(mc == 3),
                                tile_position=(0, 64),
                            )
                        nc.tensor.matmul(
                            ps[0:64, :], ones1, bj_sb[:, vs], start=False, stop=True
                        )
                        if nmv % 2 == 0:
                            nc.vector.tensor_copy(ACP[h][:, vs], ps[:])
                        else:
                            nc.scalar.copy(ACP[h][:, vs], ps[:])
                        nmv += 1

            # ---- main loop: 64 output tiles of [128, 1024], 4 per DMA ----
            with (
                tc.tile_pool(name="po", bufs=4, space="PSUM") as po,
                tc.tile_pool(name="ob", bufs=3) as ob,
            ):
                grp = None
                for k in range(64):
                    h, kk = k // 32, k % 32
                    if k % 4 == 0:
                        grp = ob.tile([128, 4 * V], FP16, tag="grp")
                    psO = po.tile([128, V], FP32, tag="psO")
                    Lk = L_sb[:, kk * 128 : (kk + 1) * 128]
                    if WIDE:
                        nc.tensor.matmul(
                            psO[:], Lk, ACP[h][:], start=True, stop=True
                        )
                    else:
                        for vh in range(2):
                            vs = slice(vh * 512, (vh + 1) * 512)
                            nc.tensor.matmul(
                                psO[:, vs], Lk, ACP[h][:, vs], start=True, stop=True
                            )
                    dst = grp[:, (k % 4) * V : (k % 4 + 1) * V]
                    if k % 2 == 0:
                        nc.vector.tensor_copy(dst, psO[:])
                    else:
                        nc.scalar.copy(dst, psO[:])
                    if k % 4 == 3:
                        g0 = (k // 4) * 512
                        nc.sync.dma_start(
                            out_d[g0 : g0 + 512, :].rearrange(
                                "(s p) v -> p s v", p=128
                            ),
                            grp[:],
                        )

    nc.compile()
    return nc


def _host_consts():
    """32 L_k selectors + identity(128) + ones row, packed [128, 34*128].
    L_k: (L_k.T @ ACP)[j,:] = C[j%64,:] + A[64h + 2k + j//64, :]."""
    Lx = np.zeros((128, 34, 128), np.float16)
    j = np.arange(128)
    u = j % 64
    hi = j // 64
    Lx[u, :32, j] = 1.0
    for kk in range(32):
        Lx[64 + 2 * kk + hi, kk, j] = 1.0
    Lx[j, 32, j] = 1.0  # identity
    Lx[0, 33, 0:U] = 1.0  # ones row
    return np.ascontiguousarray(Lx.reshape(128, 34 * 128))


def kernel(f, g, We, be, Wd, bd, Wj, bj):
    if "nc" not in _cache:
        _cache["nc"] = _build_nc()
    nc = _cache["nc"]

    c16 = lambda x: np.ascontiguousarray(np.asarray(x), dtype=np.float16)
    f16, g16 = c16(f), c16(g)
    be32 = np.asarray(be, np.float32).reshape(4, 128).T
    bd32 = np.asarray(bd, np.float32).reshape(4, 128).T
    bias8 = np.ascontiguousarray(
        np.concatenate([be32, bd32], axis=1), dtype=np.float32
    )
    shared = {
        "We": c16(We),
        "Wd": c16(Wd),
        "Wj": c16(Wj),
        "bj": c16(bj).reshape(1, V),
        "bias8": bias8,
        "Lsel": _host_consts(),
    }
    in_maps = []
    for c in range(NCORES):
        b, th = c // 2, c % 2
        in_maps.append(
            {
                "f_c": np.ascontiguousarray(f16[b, th * TC : (th + 1) * TC, :]),
                "g_c": np.ascontiguousarray(g16[b]),
                **shared,
            }
        )
    res = run_bass_kernel_spmd(nc, in_maps, list(range(NCORES)))
    kernel._last_results = res

    out = np.empty((B, T, U, V), np.float32)
    for c in range(NCORES):
        b, th = c // 2, c % 2
        out[b, th * TC : (th + 1) * TC] = res.results[c]["out"].reshape(TC, U, V)
    return out


# revision 8
# speedup vs baseline: 1.7482x; 1.0404x over previous
"""RNN-T JointNetwork kernel for 8 Trainium2 NeuronCores.

Math: out[b,t,u,:] = tanh(concat(fe[b,t], gd[b,u])) @ Wj + bj
with fe = f@We+be, gd = g@Wd+bd.

tanh is elementwise and the concat feeds one GEMM, so the joint GEMM
factorizes exactly:
    out[b,t,u,:] = A[b,t,:] + C[b,u,:]
    A = tanh(f@We+be) @ Wj[:Dm]
    C = tanh(g@Wd+bd) @ Wj[Dm:] + bj
leaving the kernel bound by the output write (fp16: 16 MB/core).

Sharding: core c owns (b = c//2, t-half = c%2) -> [128,64,V] output chunk.

On-core plan (fp16 data, fp32 PSUM):
  - f/g straight-DMA'd, transposed on PE (fp16 psum passthrough);
    feT/gdT on PE; tanh+bias on ACT -> tfT/tgT
  - ACP[h] (h = t-half of the core's 128 rows): [128,V] fp16 tile with
    partitions 0:64 = C rows (all u) and 64:128 = A rows (t in half h),
    computed into one PSUM tile with C (cols 0:64) and A (cols 64:128)
    matmuls interleaved so the two PE column-groups run concurrently.
  - main loop, per output tile k (= t-pair 2k,2k+1; 128 DRAM rows):
    psO = L_k.T @ ACP[h], where L_k is a host-built 0/1 stationary
    selecting (A row, C row) per output row -> A-broadcast + C-add in
    a single PE pass.  Movers alternate per tile: DVE / ACT copy
    PSUM->SBUF fp16.  Output DMA: 1 MB per 4 tiles.
Host: casts inputs to fp16, builds L/identity, upcasts output on gather.
"""

import sys

sys.path.insert(0, "/opt/trn_rl_repo")

import numpy as np

import concourse.bacc as bacc
import concourse.mybir as mybir
import concourse.tile as tile
from concourse.bass_utils import run_bass_kernel_spmd
B, T, U = 4, 256, 64
D = 512  # DE = DD = DM
V = 1024
TC = 128  # t rows per core
NCORES = 8
FP32 = mybir.dt.float32
FP16 = mybir.dt.float16
TANH = mybir.ActivationFunctionType.Tanh
WIDE = False  # N=1024 matmul per tile fails the walrus ISA check (1 bank/MM)

_cache = {}
_L_CONSTS = None


def _build_nc():
    nc = bacc.Bacc("TRN2", target_bir_lowering=False)

    f_d = nc.dram_tensor("f_c", [TC, D], FP16, kind="ExternalInput")
    g_d = nc.dram_tensor("g_c", [U, D], FP16, kind="ExternalInput")
    We_d = nc.dram_tensor("We", [D, D], FP16, kind="ExternalInput")
    Wd_d = nc.dram_tensor("Wd", [D, D], FP16, kind="ExternalInput")
    Wj_d = nc.dram_tensor("Wj", [2 * D, V], FP16, kind="ExternalInput")
    L_d = nc.dram_tensor("Lsel", [128, 32 * 128], FP16, kind="ExternalInput")
    c_d = nc.dram_tensor("consts", [128, 2 * 128], FP16, kind="ExternalInput")
    bias_d = nc.dram_tensor("bias8", [128, 8], FP32, kind="ExternalInput")
    bj_d = nc.dram_tensor("bj", [1, V], FP16, kind="ExternalInput")
    out_d = nc.dram_tensor("out", [TC * U, V], FP16, kind="ExternalOutput")

    with tile.TileContext(nc) as tc:
        with tc.tile_pool(name="wts", bufs=1) as wp:
            # ---- persistent SBUF ----
            f_sb = wp.tile([TC, D], FP16, tag="f")
            g_sb = wp.tile([U, D], FP16, tag="g")
            We_sb = wp.tile([128, 4 * D], FP16, tag="We")
            Wd_sb = wp.tile([128, 4 * D], FP16, tag="Wd")
            Wj_sb = wp.tile([128, 8 * V], FP16, tag="Wj")
            L_sb = wp.tile([128, 32 * 128], FP16, tag="Lsel")
            c_sb = wp.tile([128, 2 * 128], FP16, tag="consts")
            bias_sb = wp.tile([128, 8], FP32, tag="bias8")
            bj_sb = wp.tile([1, V], FP16, tag="bj")
            fT = [wp.tile([128, TC], FP16, tag=f"fT{c}", name=f"fT{c}") for c in range(4)]
            gT = [wp.tile([128, U], FP16, tag=f"gT{c}", name=f"gT{c}") for c in range(4)]
            tfT = [wp.tile([128, TC], FP16, tag=f"tfT{c}", name=f"tfT{c}") for c in range(4)]
            tgT = [wp.tile([128, U], FP16, tag=f"tgT{c}", name=f"tgT{c}") for c in range(4)]
            ACP = [wp.tile([128, V], FP16, tag=f"ACP{h}", name=f"ACP{h}") for h in range(2)]

            ident = c_sb[:, 0:128]
            ones1 = c_sb[0:1, 128 : 128 + U]

            # ---- input DMAs: few big transfers, split across HWDGE rings
            # sync ring: f, We, Wj v-half 0, L  (output DMAs follow in FIFO)
            # scalar ring: g, bias, bj, Wd, Wj v-half 1
            nc.sync.dma_start(c_sb[:], c_d[:])
            nc.sync.dma_start(f_sb[:], f_d[:])
            nc.sync.dma_start(
                We_sb[:], We_d.rearrange("(c p) m -> p c m", p=128)
            )
            nc.scalar.dma_start(g_sb[:], g_d[:])
            nc.scalar.dma_start(bias_sb[:], bias_d[:])
            nc.scalar.dma_start(bj_sb[:], bj_d[:])
            nc.scalar.dma_start(
                Wd_sb[:], Wd_d.rearrange("(c p) m -> p c m", p=128)
            )
            # Wj_sb layout: [128, c*V + v] (chunk-major); DMA by v-half
            for vh in range(2):
                eng = nc.sync if vh == 0 else nc.scalar
                eng.dma_start(
                    Wj_sb[:]
                    .rearrange("p (c v) -> p c v", c=8)[:, :, vh * 512 : (vh + 1) * 512],
                    Wj_d.rearrange("(c p) v -> p c v", p=128)[
                        :, :, vh * 512 : (vh + 1) * 512
                    ],
                )
            nc.scalar.dma_start(L_sb[:], L_d[:])

            # ---- prologue ----
            with tc.tile_pool(name="pp", bufs=4, space="PSUM") as pp:
                # transposes on PE (fp16 passthrough), copies split DVE/ACT
                for c in range(4):
                    pt = pp.tile([128, TC], FP16, tag="ppt")
                    nc.tensor.transpose(
                        pt[:], f_sb[:, c * 128 : (c + 1) * 128], ident
                    )
                    if c % 2 == 0:
                        nc.vector.tensor_copy(fT[c][:], pt[:])
                    else:
                        nc.scalar.copy(fT[c][:], pt[:])
                for c in range(4):
                    pt = pp.tile([128, U], FP16, tag="ppt")
                    nc.tensor.transpose(
                        pt[:], g_sb[:, c * 128 : (c + 1) * 128], ident[0:64, 0:64]
                    )
                    if c % 2 == 0:
                        nc.vector.tensor_copy(gT[c][:], pt[:])
                    else:
                        nc.scalar.copy(gT[c][:], pt[:])

                for mc in range(4):
                    ms = slice(mc * 128, (mc + 1) * 128)
                    ps = pp.tile([128, TC], FP32, tag="pps")
                    for dc in range(4):
                        nc.tensor.matmul(
                            ps[:],
                            We_sb[:, dc * D : (dc + 1) * D][:, ms],
                            fT[dc][:],
                            start=(dc == 0),
                            stop=(dc == 3),
                        )
                    nc.scalar.activation(
                        tfT[mc][:], ps[:], TANH, bias=bias_sb[:, mc : mc + 1]
                    )
                for mc in range(4):
                    ms = slice(mc * 128, (mc + 1) * 128)
                    ps = pp.tile([128, U], FP32, tag="pps")
                    for dc in range(4):
                        nc.tensor.matmul(
                            ps[:],
                            Wd_sb[:, dc * D : (dc + 1) * D][:, ms],
                            gT[dc][:],
                            start=(dc == 0),
                            stop=(dc == 3),
                        )
                    nc.scalar.activation(
                        tgT[mc][:], ps[:], TANH, bias=bias_sb[:, 4 + mc : 5 + mc]
                    )

                # ACP[h]: partitions 0:64 = C (all u), 64:128 = A rows of half h.
                # C matmuls hit PE col-group 0:64, A matmuls col-group 64:128 —
                # interleave them so the groups execute concurrently.
                nmv = 0
                for h in range(2):
                    hs = slice(64 * h, 64 * h + 64)
                    for vh in range(2):
                        vs = slice(vh * 512, (vh + 1) * 512)
                        Wjc = lambda c: Wj_sb[:, c * V : (c + 1) * V][:, vs]
                        ps = pp.tile([128, 512], FP32, tag="pps")
                        for mc in range(4):
                            nc.tensor.matmul(
                                ps[0:64, :],
                                tgT[mc][:],
                                Wjc(4 + mc),
                                start=(mc == 0),
                                stop=False,
                            )
                            nc.tensor.matmul(
                                ps[64:128, :],
                                tfT[mc][:, hs],
                                Wjc(mc),
                                start=(mc == 0),
                                stop=(mc == 3),
                                tile_position=(0, 64),
                            )
                        nc.tensor.matmul(
                            ps[0:64, :], ones1, bj_sb[:, vs], start=False, stop=True
                        )
                        if nmv % 2 == 0:
                            nc.vector.tensor_copy(ACP[h][:, vs], ps[:])
                        else:
                            nc.scalar.copy(ACP[h][:, vs], ps[:])
                        nmv += 1

            # ---- main loop: 64 output tiles of [128, 1024], 4 per DMA ----
            with (
                tc.tile_pool(name="po", bufs=4, space="PSUM") as po,
                tc.tile_pool(name="ob", bufs=3) as ob,
            ):
                grp = None
                for k in range(64):
                    h, kk = k // 32, k % 32
                    if k % 4 == 0:
                        grp = ob.tile([128, 4 * V], FP16, tag="grp")
                    psO = po.tile([128, V], FP32, tag="psO")
                    Lk = L_sb[:, kk * 128 : (kk + 1) * 128]
                    if WIDE:
                        nc.tensor.matmul(
                            psO[:], Lk, ACP[h][:], start=True, stop=True
                        )
                    else:
                        for vh in range(2):
                            vs = slice(vh * 512, (vh + 1) * 512)
                            nc.tensor.matmul(
                                psO[:, vs], Lk, ACP[h][:, vs], start=True, stop=True
                            )
                    dst = grp[:, (k % 4) * V : (k % 4 + 1) * V]
                    if k % 2 == 0:
                        nc.vector.tensor_copy(dst, psO[:])
                    else:
                        nc.scalar.copy(dst, psO[:])
                    if k % 4 == 3:
                        g0 = (k // 4) * 512
                        deng = nc.sync if (k // 4) % 2 == 0 else nc.scalar
                        deng.dma_start(
                            out_d[g0 : g0 + 512, :].rearrange(
                                "(s p) v -> p s v", p=128
                            ),
                            grp[:],
                        )

    nc.compile()
    return nc


def _host_consts():
    """32 L_k selectors [128, 32*128] and ident+ones [128, 2*128].
    L_k: (L_k.T @ ACP)[j,:] = C[j%64,:] + A[64h + 2k + j//64, :]."""
    Lx = np.zeros((128, 32, 128), np.float16)
    j = np.arange(128)
    u = j % 64
    hi = j // 64
    Lx[u, :, j] = 1.0
    for kk in range(32):
        Lx[64 + 2 * kk + hi, kk, j] = 1.0
    cx = np.zeros((128, 2, 128), np.float16)
    cx[j, 0, j] = 1.0  # identity
    cx[0, 1, 0:U] = 1.0  # ones row
    return (
        np.ascontiguousarray(Lx.reshape(128, 32 * 128)),
        np.ascontiguousarray(cx.reshape(128, 2 * 128)),
    )


def kernel(f, g, We, be, Wd, bd, Wj, bj):
    global _L_CONSTS
    if _L_CONSTS is None:
        _L_CONSTS = _host_consts()
    if "nc" not in _cache:
        _cache["nc"] = _build_nc()
    nc = _cache["nc"]

    c16 = lambda x: np.ascontiguousarray(np.asarray(x), dtype=np.float16)
    f16, g16 = c16(f), c16(g)
    be32 = np.asarray(be, np.float32).reshape(4, 128).T
    bd32 = np.asarray(bd, np.float32).reshape(4, 128).T
    bias8 = np.ascontiguousarray(
        np.concatenate([be32, bd32], axis=1), dtype=np.float32
    )
    shared = {
        "We": c16(We),
        "Wd": c16(Wd),
        "Wj": c16(Wj),
        "bj": c16(bj).reshape(1, V),
        "bias8": bias8,
        "Lsel": _L_CONSTS[0],
        "consts": _L_CONSTS[1],
    }
    in_maps = []
    for c in range(NCORES):
        b, th = c // 2, c % 2
        in_maps.append(
            {
                "f_c": np.ascontiguousarray(f16[b, th * TC : (th + 1) * TC, :]),
                "g_c": np.ascontiguousarray(g16[b]),
                **shared,
            }
        )
    res = run_bass_kernel_spmd(nc, in_maps, list(range(NCORES)))
    kernel._last_results = res

    out = np.empty((B, T, U, V), np.float32)
    for c in range(NCORES):
        b, th = c // 2, c % 2
        out[b, th * TC : (th + 1) * TC] = res.results[c]["out"].reshape(TC, U, V)
    return out
